# revision 13
# baseline (speedup 1.0000x reference)
"""DividedSpaceTimeAttention Trainium2 kernel (8 NeuronCores, Bass/Tile).

Problem shapes (hardcoded): x (B=2, C=256, T=16, H=32, W=32) fp32.

Sharding: core i owns batch b=i//4 and frame-group fg=i%4 (4 frames).
Each core receives x[b] with frames ROTATED so its own 4 frames come
first (temporal attention sums over all keys, so key order is
irrelevant; frames are otherwise independent), computes its 4 frames of
the final output completely, and the host reassembles.

Per-core pipeline:
  T1: temporal q/k/v projections in token-major layout [tok, C] (bf16
      matmuls with per-tile stationary x), k/v for all 16 frames, q for
      the 4 owned frames; DRAM scratch.
  T2: temporal axial attention per 128-location block on the Vector
      engine: q*k products (broadcast APs) + segmented reduces, exp on
      the Scalar engine (no max subtraction: |scores| ~ 1), AV the same
      way; token-major output to DRAM.
  T3: temporal output projection (wf_t) with DMA-transposed bf16 rhs.
  S:  spatial attention per owned frame: GroupNorm (bn_stats + PE 0/1
      group-reduce/broadcast matmuls), q/k/vT projections, transposed
      scores S'[kj,qi] (so softmax sum runs on partitions via a
      ones-matmul and AV needs no transposes), exp without max
      subtraction, AV, o-projection, + x residual + temporal branch.

All big matmuls run in bf16 (full PE rate, FWL weight loads);
the tiny GroupNorm stats matmuls stay plain fp32. Walrus in this
build accepts at most one semaphore wait per instruction, so a post-pass
splits multi-wait instructions (see _split_multi_waits).
"""

import sys

sys.path.insert(0, "/opt/trn_rl_repo")

import numpy as np

import concourse.bass as bass
from concourse import mybir
from concourse.bass_utils import run_bass_kernel_spmd
from concourse.tile import TileContext

B, C, T, H, W = 2, 256, 16, 32, 32
NH = 8
DK = C // NH  # 32
GROUPS = 32
CPG = C // GROUPS  # 8
EPS = 1e-6
HWN = H * W  # 1024
TOWN = 4  # owned frames per core
NTOK = T * HWN  # 16384 tokens per batch
FD = TOWN * HWN  # 4096 owned tokens
P = 128
NLB = HWN // P  # 8 location blocks
FP32 = mybir.dt.float32
FP32R = mybir.dt.float32r
BF16 = mybir.dt.bfloat16

_CACHE = {}


def _split_multi_waits(nc):
    """This walrus build rejects >1 sync wait per instruction
    ("Too many sync wait commands"); hoist extras onto same-engine NOPs
    inserted immediately before the instruction."""
    cur_bb = nc.cur_bb.bb if hasattr(nc.cur_bb, "bb") else nc.cur_bb
    for f in nc.m.functions:
        for bb in f.blocks:
            insts = bb.instructions
            i = 0
            while i < len(insts):
                inst = insts[i]
                si = inst.sync_info
                waits = list(si.on_wait) if si and si.on_wait else []
                if len(waits) > 1:
                    nops = []
                    for w in waits[:-1]:
                        nop = nc.engines[inst.engine].nop(hint="waitsplit").ins
                        host = cur_bb.instructions
                        assert host[-1].name == nop.name
                        host.pop()
                        nop.sync_info = mybir.SyncInfo(on_wait=[w], on_update=[])
                        nops.append(nop)
                    inst.sync_info = mybir.SyncInfo(
                        on_wait=[waits[-1]],
                        on_update=list(si.on_update) if si.on_update else [])
                    for j, nop in enumerate(nops):
                        insts.insert(i + j, nop)
                    i += len(nops)
                i += 1


def _build_bass():
    nc = bass.Bass()

    xin = nc.declare_dram_parameter("x", [C, NTOK], FP32, isOutput=False)
    wqsT = nc.declare_dram_parameter("wqsT", [C, C], FP32, isOutput=False)
    wksT = nc.declare_dram_parameter("wksT", [C, C], FP32, isOutput=False)
    wvsT = nc.declare_dram_parameter("wvsT", [C, C], FP32, isOutput=False)
    wosT = nc.declare_dram_parameter("wosT", [C, C], FP32, isOutput=False)
    wqkv = nc.declare_dram_parameter("wqkv", [C, 3 * C], FP32, isOutput=False)
    wft = nc.declare_dram_parameter("wft", [C, C], FP32, isOutput=False)
    vecs = nc.declare_dram_parameter("vecs", [C, 8], FP32, isOutput=False)
    # vecs cols: 0 gn_gamma, 1 gn_beta, 2 bq_s, 3 bk_s, 4 bv_s, 5 bo_s, 6 bf_t
    gsum = nc.declare_dram_parameter("gsum", [P, 16], FP32, isOutput=False)
    gbc = nc.declare_dram_parameter("gbc", [16, P], FP32, isOutput=False)

    out = nc.declare_dram_parameter("out", [TOWN, C, HWN], FP32, isOutput=True)

    k_t = nc.dram_tensor("k_t_scr", [NTOK, C], BF16)
    v_t = nc.dram_tensor("v_t_scr", [NTOK, C], BF16)
    q_t = nc.dram_tensor("q_t_scr", [FD, C], BF16)
    o_t = nc.dram_tensor("o_t_scr", [FD, C], BF16)
    tt_t = nc.dram_tensor("tt_scr", [C, FD], FP32)

    inv_sqrt_dk = 1.0 / float(np.sqrt(DK))
    c_scale = 1.0 / float(np.sqrt(C))
    AF = mybir.ActivationFunctionType
    OP = mybir.AluOpType

    with TileContext(nc) as tc:
        with tc.tile_pool(name="consts", bufs=1) as consts:
            # ---- persistent constants ----
            def loadw16(dram, tag):
                ts = []
                for ch in range(2):
                    t32 = consts.tile([P, C], FP32, tag="wstage", name=f"{tag}32_{ch}")
                    nc.sync.dma_start(out=t32, in_=dram[ch * P:(ch + 1) * P, :])
                    t16 = consts.tile([P, C], BF16, tag=f"{tag}16_{ch}",
                                      name=f"{tag}16_{ch}")
                    nc.vector.tensor_copy(t16, t32)
                    ts.append(t16)
                return ts

            wqsT_s = loadw16(wqsT, "wqs")
            wksT_s = loadw16(wksT, "wks")
            wvsT_s = loadw16(wvsT, "wvs")
            wosT_s = loadw16(wosT, "wos")
            wqkv16 = []
            for ch in range(2):
                t32 = consts.tile([P, 3 * C], FP32, tag=f"wqkv32{ch}")
                nc.sync.dma_start(out=t32, in_=wqkv[ch * P:(ch + 1) * P, :])
                t16 = consts.tile([P, 3 * C], BF16, tag=f"wqkv16{ch}")
                nc.vector.tensor_copy(t16, t32)
                wqkv16.append(t16)
            wft16 = []
            for ch in range(2):
                t32 = consts.tile([P, C], FP32, tag=f"wft32{ch}")
                nc.sync.dma_start(out=t32, in_=wft[ch * P:(ch + 1) * P, :])
                t16 = consts.tile([P, C], BF16, tag=f"wft16{ch}")
                nc.vector.tensor_copy(t16, t32)
                wft16.append(t16)
            vec_s = []
            for ch in range(2):
                t = consts.tile([P, 8], FP32, tag=f"vec{ch}")
                nc.sync.dma_start(out=t, in_=vecs[ch * P:(ch + 1) * P, :])
                vec_s.append(t)
            gsum_s = consts.tile([P, 16], FP32)
            nc.sync.dma_start(out=gsum_s, in_=gsum[:, :])
            gbc_s = consts.tile([16, P], FP32)
            nc.sync.dma_start(out=gbc_s, in_=gbc[:, :])
            ones_col = consts.tile([P, 1], BF16)
            nc.vector.memset(ones_col, 1.0)
            ones_row = consts.tile([1, P], FP32)
            nc.vector.memset(ones_row, 1.0)

            # ============ T1: temporal q/k/v projections (token-major) ========
            with tc.tile_pool(name="t1", bufs=3) as t1p, \
                 tc.tile_pool(name="t1ps", bufs=3, space="PSUM") as t1ps:
                GW = 512  # tokens per group (4 tiles of 128)
                for g in range(NTOK // GW):
                    own = g < (FD // GW)
                    ncols = 3 * C if own else 2 * C
                    xt = t1p.tile([P, 2, GW], FP32, tag="xt")
                    for ch in range(2):
                        nc.sync.dma_start(
                            out=xt[:, ch, :],
                            in_=xin[ch * P:(ch + 1) * P, g * GW:(g + 1) * GW])
                    xt16 = t1p.tile([P, 2, GW], BF16, tag="xt16")
                    nc.scalar.activation(xt16, xt, AF.Copy)
                    kb = t1p.tile([P, 4, C], BF16, tag="kb")
                    vb = t1p.tile([P, 4, C], BF16, tag="vb")
                    qb = t1p.tile([P, 4, C], BF16, tag="qb", name="qb") if own else None
                    for s in range(4):
                        pp = t1ps.tile([P, 3 * C], FP32, tag="t1pp")
                        segs = ([(0, 512), (512, ncols)] if ncols > 512
                                else [(0, 512)])
                        for s0, s1 in segs:
                            for ch in range(2):
                                nc.tensor.matmul(
                                    pp[:, s0:s1],
                                    xt16[:, ch, s * P:(s + 1) * P],
                                    wqkv16[ch][:, s0:s1],
                                    start=(ch == 0), stop=(ch == 1))
                        nc.scalar.activation(kb[:, s], pp[:, 0:C], AF.Copy)
                        nc.vector.tensor_copy(vb[:, s], pp[:, C:2 * C])
                        if own:
                            nc.scalar.activation(qb[:, s], pp[:, 2 * C:3 * C],
                                                 AF.Copy, scale=inv_sqrt_dk)
                    kv_dst = k_t[g * GW:(g + 1) * GW, :]
                    nc.sync.dma_start(
                        out=kv_dst.rearrange("(s p) c -> p s c", s=4), in_=kb)
                    vv_dst = v_t[g * GW:(g + 1) * GW, :]
                    nc.sync.dma_start(
                        out=vv_dst.rearrange("(s p) c -> p s c", s=4), in_=vb)
                    if own:
                        qq_dst = q_t[g * GW:(g + 1) * GW, :]
                        nc.sync.dma_start(
                            out=qq_dst.rearrange("(s p) c -> p s c", s=4), in_=qb)

            # ============ T2: temporal attention (DVE) ============
            with tc.tile_pool(name="t2", bufs=2) as t2p:
                k_view = k_t.rearrange("(t lb p) c -> t lb p c", t=T, lb=NLB)
                v_view = v_t.rearrange("(t lb p) c -> t lb p c", t=T, lb=NLB)
                q_view = q_t.rearrange("(t lb p) c -> t lb p c", t=TOWN, lb=NLB)
                o_view = o_t.rearrange("(t lb p) c -> t lb p c", t=TOWN, lb=NLB)
                for lb in range(NLB):
                    kk = t2p.tile([P, T, C], BF16, tag="kk")
                    vv = t2p.tile([P, T, C], BF16, tag="vv")
                    qq = t2p.tile([P, TOWN, C], BF16, tag="qq")
                    nc.sync.dma_start(out=kk, in_=k_view[:, lb].transpose([1, 0, 2]))
                    nc.sync.dma_start(out=vv, in_=v_view[:, lb].transpose([1, 0, 2]))
                    nc.sync.dma_start(out=qq, in_=q_view[:, lb].transpose([1, 0, 2]))
                    # scores: prod[p, ti, tj, (n d)] = qq[p, ti, (n d)] * kk[p, tj, (n d)]
                    prod = t2p.tile([P, TOWN, T, C], BF16, tag="prodX", name="prod")
                    for ti in range(TOWN):
                        q_sl = qq[:, ti, :]  # [p, 256]
                        in0 = bass.AP(tensor=q_sl.tensor, offset=q_sl.offset,
                                      ap=[q_sl.ap[0], [0, T], q_sl.ap[1]])
                        nc.vector.tensor_tensor(out=prod[:, ti], in0=in0, in1=kk,
                                                op=OP.mult)
                    # S[p, (ti tj n)] = sum_d prod ; view prod as [p,(ti tj n),d]
                    s_sc = t2p.tile([P, TOWN, T, NH], BF16, tag="s_sc")
                    with nc.allow_low_precision(reason="scores are O(0.1)"):
                        nc.vector.tensor_reduce(
                            out=s_sc.rearrange("p a b n -> p (a b n)"),
                            in_=prod.rearrange("p a b (n d) -> p (a b n) d", n=NH),
                            axis=mybir.AxisListType.X, op=OP.add)
                    e_sc = t2p.tile([P, TOWN, T, NH], BF16, tag="e_sc")
                    nc.scalar.activation(e_sc, s_sc, AF.Exp)
                    # z[p, ti, n] = sum_tj e ; e viewed [p, (ti n), tj]
                    z = t2p.tile([P, TOWN, NH], FP32, tag="z")
                    for ti in range(TOWN):
                        e_sl = e_sc[:, ti]  # [p, T, NH]
                        nc.vector.tensor_reduce(
                            out=z[:, ti], in_=e_sl.transpose([0, 2, 1]),
                            axis=mybir.AxisListType.X, op=OP.add)
                    zr = t2p.tile([P, TOWN, NH], FP32, tag="zr")
                    nc.vector.reciprocal(out=zr.rearrange("p a n -> p (a n)"),
                                         in_=z.rearrange("p a n -> p (a n)"))
                    # AV: prod2[p, ti, (n d), tj] = e[p, ti, tj, n] * vv[p, tj, (n d)]
                    prod2 = t2p.tile([P, TOWN, C, T], BF16, tag="prodX", name="prod2")
                    vvv = vv.rearrange("p tj (n d) -> p tj n d", n=NH)
                    v_b = bass.AP(tensor=vvv.tensor, offset=vvv.offset,
                                  ap=[vvv.ap[0], vvv.ap[2], vvv.ap[3], vvv.ap[1]])
                    for ti in range(TOWN):
                        e_sl = e_sc[:, ti].rearrange("p tj n -> p n tj")
                        in0 = bass.AP(tensor=e_sl.tensor, offset=e_sl.offset,
                                      ap=[e_sl.ap[0], e_sl.ap[1], [0, DK],
                                          e_sl.ap[2]])
                        nc.vector.tensor_tensor(
                            out=prod2[:, ti].rearrange("p (n d) tj -> p n d tj",
                                                       n=NH),
                            in0=in0, in1=v_b, op=OP.mult)
                    oun = t2p.tile([P, TOWN, C], BF16, tag="oun")
                    with nc.allow_low_precision(reason="attn out is O(0.3)"):
                        nc.vector.tensor_reduce(
                            out=oun.rearrange("p a c -> p (a c)"),
                            in_=prod2.rearrange("p a c t -> p (a c) t"),
                            axis=mybir.AxisListType.X, op=OP.add)
                    # normalize by z (broadcast over d) and cast bf16
                    ob = t2p.tile([P, TOWN, C], BF16, tag="ob")
                    zv = zr  # [p, ti, n]
                    in1 = bass.AP(tensor=zv.tensor, offset=zv.offset,
                                  ap=[zv.ap[0], zv.ap[1], zv.ap[2], [0, DK]])
                    nc.vector.tensor_tensor(
                        out=ob.rearrange("p a (n d) -> p a n d", n=NH),
                        in0=oun.rearrange("p a (n d) -> p a n d", n=NH),
                        in1=in1, op=OP.mult)
                    nc.sync.dma_start(out=o_view[:, lb].transpose([1, 0, 2]), in_=ob)

            # ============ T3: temporal f-projection ============
            with tc.tile_pool(name="t3", bufs=3) as t3p, \
                 tc.tile_pool(name="t3ps", bufs=2, space="PSUM") as t3ps:
                for nt in range(FD // 512):
                    otr = [t3p.tile([P, 512], BF16, tag=f"otr{ch}", name=f"otr{ch}") for ch in range(2)]
                    for ch in range(2):
                        nc.sync.dma_start_transpose(
                            out=otr[ch],
                            in_=o_t[nt * 512:(nt + 1) * 512, ch * P:(ch + 1) * P])
                    for co in range(2):
                        ttp = t3ps.tile([P, 512], FP32, tag="ttp")
                        for ch in range(2):
                            nc.tensor.matmul(ttp, wft16[ch][:, co * P:(co + 1) * P],
                                             otr[ch], start=(ch == 0), stop=(ch == 1))
                        ttsb = t3p.tile([P, 512], FP32, tag="ttsb")
                        nc.scalar.activation(ttsb, ttp, AF.Identity,
                                             bias=vec_s[co][:, 6:7])
                        nc.sync.dma_start(
                            out=tt_t[co * P:(co + 1) * P, nt * 512:(nt + 1) * 512],
                            in_=ttsb)

            # ============ S: spatial attention per owned frame ============
            with tc.tile_pool(name="sp", bufs=2) as spp, \
                 tc.tile_pool(name="spb", bufs=1) as spb, \
                 tc.tile_pool(name="sps", bufs=2, space="PSUM") as sps, \
                 tc.tile_pool(name="spsb", bufs=1, space="PSUM") as spsb:
                for fr in range(TOWN):
                    col0 = fr * HWN
                    xf = [spp.tile([P, HWN], FP32, tag=f"xf{ch}", name=f"xf{ch}") for ch in range(2)]
                    for ch in range(2):
                        nc.sync.dma_start(
                            out=xf[ch], in_=xin[ch * P:(ch + 1) * P, col0:col0 + HWN])
                    # ---- GroupNorm stats ----
                    stats = spp.tile([P, 2, 2, 6], FP32, tag="stats")
                    mv = spp.tile([P, 2, 2], FP32, tag="mv")
                    for ch in range(2):
                        for sb_ in range(2):
                            nc.vector.bn_stats(out=stats[:, ch, sb_],
                                               in_=xf[ch][:, sb_ * 512:(sb_ + 1) * 512])
                        nc.vector.bn_aggr(out=mv[:, ch], in_=stats[:, ch])
                    # per-channel [mean, E[x^2]]
                    ex2 = spp.tile([P, 2, 2], FP32, tag="ex2")
                    for ch in range(2):
                        nc.vector.tensor_tensor(out=ex2[:, ch, 1:2],
                                                in0=mv[:, ch, 0:1], in1=mv[:, ch, 0:1],
                                                op=OP.mult)
                        nc.vector.tensor_tensor(out=ex2[:, ch, 1:2],
                                                in0=mv[:, ch, 1:2], in1=ex2[:, ch, 1:2],
                                                op=OP.add)
                        nc.vector.tensor_copy(ex2[:, ch, 0:1], mv[:, ch, 0:1])
                    # group sums via 0/1 matmul: [16, 2] per chunk
                    gstat = [sps.tile([16, 2], FP32, tag="pj", name=f"gstat{ch}")
                             for ch in range(2)]
                    for ch in range(2):
                        nc.tensor.matmul(gstat[ch], gsum_s, ex2[:, ch],
                                         start=True, stop=True)
                    gsb = spp.tile([16, 2, 2], FP32, tag="gsb")
                    for ch in range(2):
                        nc.vector.tensor_scalar_mul(gsb[:, ch], gstat[ch], 1.0 / CPG)
                    # catstat[:, ch, 0] = mean_g ; [:, ch, 1] = rstd_g
                    catstat = spp.tile([16, 2, 2], FP32, tag="catstat")
                    for ch in range(2):
                        nc.vector.tensor_copy(catstat[:, ch, 0:1], gsb[:, ch, 0:1])
                        # var = E[x^2] - mean^2
                        nc.vector.tensor_tensor(out=catstat[:, ch, 1:2],
                                                in0=gsb[:, ch, 0:1],
                                                in1=gsb[:, ch, 0:1], op=OP.mult)
                        nc.vector.tensor_tensor(out=catstat[:, ch, 1:2],
                                                in0=gsb[:, ch, 1:2],
                                                in1=catstat[:, ch, 1:2],
                                                op=OP.subtract)
                    # rstd = 1/sqrt(var + eps)
                    nc.vector.tensor_scalar_add(
                        catstat[:, :, 1], catstat[:, :, 1], EPS)
                    nc.scalar.activation(
                        catstat[:, :, 1], catstat[:, :, 1], AF.Sqrt)
                    nc.vector.reciprocal(out=catstat[:, :, 1], in_=catstat[:, :, 1])
                    # broadcast groups -> channels via 0/1 matmul [128, 2] per chunk
                    bc = [sps.tile([P, 2], FP32, tag="pj", name=f"bc{ch}")
                          for ch in range(2)]
                    for ch in range(2):
                        nc.tensor.matmul(bc[ch], gbc_s, catstat[:, ch],
                                         start=True, stop=True)
                    # scale_c = rstd*gamma ; bias_c = beta - mean*scale
                    scb = spp.tile([P, 2, 2], FP32, tag="scb")
                    for ch in range(2):
                        nc.vector.tensor_tensor(out=scb[:, ch, 0:1],
                                                in0=bc[ch][:, 1:2],
                                                in1=vec_s[ch][:, 0:1], op=OP.mult)
                        nc.vector.tensor_tensor(out=scb[:, ch, 1:2],
                                                in0=bc[ch][:, 0:1],
                                                in1=scb[:, ch, 0:1], op=OP.mult)
                        nc.vector.tensor_tensor(out=scb[:, ch, 1:2],
                                                in0=vec_s[ch][:, 1:2],
                                                in1=scb[:, ch, 1:2], op=OP.subtract)
                    hf = [spp.tile([P, HWN], BF16, tag=f"hf{ch}", name=f"hf{ch}") for ch in range(2)]
                    for ch in range(2):
                        nc.vector.tensor_scalar(
                            out=hf[ch], in0=xf[ch], scalar1=scb[:, ch, 0:1],
                            scalar2=scb[:, ch, 1:2], op0=OP.mult, op1=OP.add)
                    # ---- q, k projections [co, hw] ----
                    qs = [spp.tile([P, HWN], BF16, tag=f"qs{ch}", name=f"qs{ch}") for ch in range(2)]
                    ks = [spp.tile([P, HWN], BF16, tag=f"ks{ch}", name=f"ks{ch}") for ch in range(2)]
                    for co in range(2):
                        for half in range(2):
                            nsl = slice(half * 512, (half + 1) * 512)
                            qp = sps.tile([P, 512], FP32, tag="pj")
                            kp = sps.tile([P, 512], FP32, tag="pj")
                            for ch in range(2):
                                nc.tensor.matmul(
                                    qp, wqsT_s[ch][:, co * P:(co + 1) * P],
                                    hf[ch][:, nsl],
                                    start=(ch == 0), stop=(ch == 1))
                                nc.tensor.matmul(
                                    kp, wksT_s[ch][:, co * P:(co + 1) * P],
                                    hf[ch][:, nsl],
                                    start=(ch == 0), stop=(ch == 1))
                            nc.scalar.activation(qs[co][:, nsl], qp, AF.Identity,
                                                 bias=vec_s[co][:, 2:3])
                            nc.scalar.activation(ks[co][:, nsl], kp, AF.Identity,
                                                 bias=vec_s[co][:, 3:4])
                    # ---- vT[hw, co] (bias bv folded in after AV) ----
                    vT = spb.tile([P, 8, C], BF16, tag="vT")
                    for hwt in range(8):
                        vp = sps.tile([P, C], FP32, tag="pj")
                        for ch in range(2):
                            nc.tensor.matmul(
                                vp, hf[ch][:, hwt * P:(hwt + 1) * P],
                                wvsT_s[ch],
                                start=(ch == 0), stop=(ch == 1))
                        nc.scalar.activation(vT[:, hwt], vp, AF.Copy)
                    # ---- scores S'[kj, qi] -> exp ----
                    ebuf = spb.tile([P, 8, HWN], BF16, tag="ebuf")
                    for kjt in range(8):
                        sp_ = sps.tile([P, HWN], FP32, tag="big")
                        for half in range(2):
                            nsl = slice(half * 512, (half + 1) * 512)
                            for ch in range(2):
                                nc.tensor.matmul(
                                    sp_[:, nsl],
                                    ks[ch][:, kjt * P:(kjt + 1) * P],
                                    qs[ch][:, nsl],
                                    start=(ch == 0), stop=(ch == 1))
                        nc.scalar.activation(ebuf[:, kjt], sp_, AF.Exp, scale=c_scale)
                    # ---- rowsum over kj via ones-matmul ----
                    rs = spsb.tile([1, HWN], FP32, tag="rs")
                    for half in range(2):
                        nsl = slice(half * 512, (half + 1) * 512)
                        for kjt in range(8):
                            nc.tensor.matmul(rs[:, nsl], ones_col, ebuf[:, kjt, nsl],
                                             start=(kjt == 0), stop=(kjt == 7))
                    rsr = spp.tile([1, HWN], FP32, tag="rsr")
                    nc.vector.reciprocal(out=rsr, in_=rs)
                    # broadcast rowsum recip to all partitions via K=1 matmul
                    rsbp = sps.tile([P, HWN], FP32, tag="big")
                    for half in range(2):
                        nsl = slice(half * 512, (half + 1) * 512)
                        nc.tensor.matmul(rsbp[:, nsl], ones_row, rsr[:, nsl],
                                         start=True, stop=True)
                    rsb = spp.tile([P, HWN], FP32, tag="rsb")
                    nc.scalar.activation(rsb, rsbp, AF.Copy)
                    # ---- AV + normalize + bv ----
                    attn_out = [spp.tile([P, HWN], BF16, tag=f"ao{ch}", name=f"ao{ch}")
                                for ch in range(2)]
                    for co in range(2):
                        avp = sps.tile([P, HWN], FP32, tag="big")
                        for half in range(2):
                            nsl = slice(half * 512, (half + 1) * 512)
                            for kjt in range(8):
                                nc.tensor.matmul(avp[:, nsl],
                                                 vT[:, kjt, co * P:(co + 1) * P],
                                                 ebuf[:, kjt, nsl],
                                                 start=(kjt == 0), stop=(kjt == 7))
                        nc.vector.tensor_tensor(out=attn_out[co], in0=avp, in1=rsb,
                                                op=OP.mult)
                        nc.vector.tensor_scalar_add(attn_out[co], attn_out[co],
                                                    vec_s[co][:, 4:5])
                    # ---- o-proj + residual + temporal ----
                    for co in range(2):
                        for half in range(2):
                            nsl = slice(half * 512, (half + 1) * 512)
                            op_ = sps.tile([P, 512], FP32, tag="pj")
                            for ch in range(2):
                                nc.tensor.matmul(
                                    op_,
                                    wosT_s[ch][:, co * P:(co + 1) * P],
                                    attn_out[ch][:, nsl],
                                    start=(ch == 0), stop=(ch == 1))
                            fin = spp.tile([P, 512], FP32, tag="fin")
                            nc.scalar.activation(fin, op_, AF.Identity,
                                                 bias=vec_s[co][:, 5:6])
                            nc.vector.tensor_tensor(out=fin, in0=fin,
                                                    in1=xf[co][:, nsl], op=OP.add)
                            ttl = spp.tile([P, 512], FP32, tag="ttl")
                            nc.sync.dma_start(
                                out=ttl,
                                in_=tt_t[co * P:(co + 1) * P,
                                         col0 + half * 512:col0 + (half + 1) * 512])
                            nc.vector.tensor_tensor(out=fin, in0=fin, in1=ttl,
                                                    op=OP.add)
                            nc.sync.dma_start(
                                out=out[fr, co * P:(co + 1) * P, nsl], in_=fin)

    _split_multi_waits(nc)
    return nc


def _get_nc():
    if "nc" not in _CACHE:
        _CACHE["nc"] = _build_bass()
    return _CACHE["nc"]


def _make_in_maps(inputs):
    x = np.asarray(inputs["x"], dtype=np.float32)  # (B, C, T, H, W)
    g = {k: np.asarray(v, dtype=np.float32) for k, v in inputs.items()}

    wqkv = np.concatenate([g["wk_t"], g["wv_t"], g["wq_t"]], axis=1)  # [C, 3C]
    vecs = np.stack([g["gn_gamma"], g["gn_beta"], g["bq_s"], g["bk_s"],
                     g["bv_s"], g["bo_s"], g["bf_t"],
                     np.zeros(C, np.float32)], axis=1)  # [C, 8]
    pidx = np.arange(P)
    gsum = (pidx[:, None] // CPG == np.arange(16)[None, :]).astype(np.float32)
    gbc = np.ascontiguousarray(gsum.T)

    shared = dict(
        wqsT=np.ascontiguousarray(g["wq_s"].T), wksT=np.ascontiguousarray(g["wk_s"].T),
        wvsT=np.ascontiguousarray(g["wv_s"].T), wosT=np.ascontiguousarray(g["wo_s"].T),
        wqkv=np.ascontiguousarray(wqkv), wft=np.ascontiguousarray(g["wf_t"]),
        vecs=np.ascontiguousarray(vecs), gsum=gsum, gbc=gbc,
    )
    in_maps = []
    for core in range(8):
        b, fg = core // 4, core % 4
        xb = np.roll(x[b], -TOWN * fg, axis=1)  # (C, T, H, W), owned frames first
        m = dict(shared)
        m["x"] = np.ascontiguousarray(xb.reshape(C, NTOK))
        in_maps.append(m)
    return in_maps


def kernel(**inputs):
    in_maps = _make_in_maps(inputs)
    nc = _get_nc()
    res = run_bass_kernel_spmd(nc, in_maps, core_ids=list(range(8)))

    out = np.empty((B, C, T, H, W), dtype=np.float32)
    for core in range(8):
        b, fg = core // 4, core % 4
        o = res.results[core]["out"]  # [TOWN, C, HWN]
        out[b, :, TOWN * fg:TOWN * (fg + 1)] = (
            o.transpose(1, 0, 2).reshape(C, TOWN, H, W))
    return out


# revision 17
# speedup vs baseline: 1.0139x; 1.0139x over previous
"""DividedSpaceTimeAttention Trainium2 kernel (8 NeuronCores, Bass/Tile).

Problem shapes (hardcoded): x (B=2, C=256, T=16, H=32, W=32) fp32.

Sharding: core i owns batch b=i//4 and frame-group fg=i%4 (4 frames).
Each core receives x[b] with frames ROTATED so its own 4 frames come
first (temporal attention sums over all keys, so key order is
irrelevant; frames are otherwise independent), computes its 4 frames of
the final output completely, and the host reassembles.

Per-core pipeline:
  T1: temporal q/k/v projections in token-major layout [tok, C] (bf16
      matmuls with per-tile stationary x), k/v for all 16 frames, q for
      the 4 owned frames; DRAM scratch.
  T2: temporal axial attention per 128-location block on the Vector
      engine: q*k products (broadcast APs) + segmented reduces, exp on
      the Scalar engine (no max subtraction: |scores| ~ 1), AV the same
      way; token-major output to DRAM.
  T3: temporal output projection (wf_t) with DMA-transposed bf16 rhs.
  S:  spatial attention per owned frame: GroupNorm (bn_stats + PE 0/1
      group-reduce/broadcast matmuls), q/k/vT projections, transposed
      scores S'[kj,qi] (so softmax sum runs on partitions via a
      ones-matmul and AV needs no transposes), exp without max
      subtraction, AV, o-projection, + x residual + temporal branch.

All big matmuls run in bf16 (full PE rate, FWL weight loads);
the tiny GroupNorm stats matmuls stay plain fp32. Walrus in this
build accepts at most one semaphore wait per instruction, so a post-pass
splits multi-wait instructions (see _split_multi_waits).
"""

import sys

sys.path.insert(0, "/opt/trn_rl_repo")

import numpy as np

import concourse.bass as bass
from concourse import mybir
from concourse.bass_utils import run_bass_kernel_spmd
from concourse.tile import TileContext

B, C, T, H, W = 2, 256, 16, 32, 32
NH = 8
DK = C // NH  # 32
GROUPS = 32
CPG = C // GROUPS  # 8
EPS = 1e-6
HWN = H * W  # 1024
TOWN = 4  # owned frames per core
NTOK = T * HWN  # 16384 tokens per batch
FD = TOWN * HWN  # 4096 owned tokens
P = 128
NLB = HWN // P  # 8 location blocks
FP32 = mybir.dt.float32
FP32R = mybir.dt.float32r
BF16 = mybir.dt.bfloat16

_CACHE = {}


def _split_multi_waits(nc):
    """This walrus build rejects >1 sync wait per instruction
    ("Too many sync wait commands"); hoist extras onto same-engine NOPs
    inserted immediately before the instruction."""
    cur_bb = nc.cur_bb.bb if hasattr(nc.cur_bb, "bb") else nc.cur_bb
    for f in nc.m.functions:
        for bb in f.blocks:
            insts = bb.instructions
            i = 0
            while i < len(insts):
                inst = insts[i]
                si = inst.sync_info
                waits = list(si.on_wait) if si and si.on_wait else []
                if len(waits) > 1:
                    nops = []
                    for w in waits[:-1]:
                        nop = nc.engines[inst.engine].nop(hint="waitsplit").ins
                        host = cur_bb.instructions
                        assert host[-1].name == nop.name
                        host.pop()
                        nop.sync_info = mybir.SyncInfo(on_wait=[w], on_update=[])
                        nops.append(nop)
                    inst.sync_info = mybir.SyncInfo(
                        on_wait=[waits[-1]],
                        on_update=list(si.on_update) if si.on_update else [])
                    for j, nop in enumerate(nops):
                        insts.insert(i + j, nop)
                    i += len(nops)
                i += 1


def _build_bass():
    nc = bass.Bass()

    xin = nc.declare_dram_parameter("x", [C, NTOK], FP32, isOutput=False)
    wqsT = nc.declare_dram_parameter("wqsT", [C, C], FP32, isOutput=False)
    wksT = nc.declare_dram_parameter("wksT", [C, C], FP32, isOutput=False)
    wvsT = nc.declare_dram_parameter("wvsT", [C, C], FP32, isOutput=False)
    wosT = nc.declare_dram_parameter("wosT", [C, C], FP32, isOutput=False)
    wqkv = nc.declare_dram_parameter("wqkv", [C, 3 * C], FP32, isOutput=False)
    wft = nc.declare_dram_parameter("wft", [C, C], FP32, isOutput=False)
    vecs = nc.declare_dram_parameter("vecs", [C, 8], FP32, isOutput=False)
    # vecs cols: 0 gn_gamma, 1 gn_beta, 2 bq_s, 3 bk_s, 4 bv_s, 5 bo_s, 6 bf_t
    gsum = nc.declare_dram_parameter("gsum", [P, 16], FP32, isOutput=False)
    gbc = nc.declare_dram_parameter("gbc", [16, P], FP32, isOutput=False)

    out = nc.declare_dram_parameter("out", [TOWN, C, HWN], FP32, isOutput=True)

    k_t = nc.dram_tensor("k_t_scr", [NTOK, C], BF16)
    v_t = nc.dram_tensor("v_t_scr", [NTOK, C], BF16)
    q_t = nc.dram_tensor("q_t_scr", [FD, C], BF16)
    o_t = nc.dram_tensor("o_t_scr", [FD, C], BF16)
    tt_t = nc.dram_tensor("tt_scr", [C, FD], FP32)
    s_scr = nc.dram_tensor("s_scr", [TOWN, C, HWN], FP32)

    inv_sqrt_dk = 1.0 / float(np.sqrt(DK))
    c_scale = 1.0 / float(np.sqrt(C))
    AF = mybir.ActivationFunctionType
    OP = mybir.AluOpType

    with TileContext(nc, pool_alloc_mode="queue") as tc:
        with tc.tile_pool(name="consts", bufs=1) as consts:
            # ---- persistent constants ----
            def loadw16(dram, tag):
                ts = []
                for ch in range(2):
                    t32 = consts.tile([P, C], FP32, tag="wstage", name=f"{tag}32_{ch}")
                    nc.sync.dma_start(out=t32, in_=dram[ch * P:(ch + 1) * P, :])
                    t16 = consts.tile([P, C], BF16, tag=f"{tag}16_{ch}",
                                      name=f"{tag}16_{ch}")
                    nc.vector.tensor_copy(t16, t32)
                    ts.append(t16)
                return ts

            wqsT_s = loadw16(wqsT, "wqs")
            wksT_s = loadw16(wksT, "wks")
            wvsT_s = loadw16(wvsT, "wvs")
            wosT_s = loadw16(wosT, "wos")
            wqkv16 = []
            for ch in range(2):
                t32 = consts.tile([P, 3 * C], FP32, tag=f"wqkv32{ch}")
                nc.sync.dma_start(out=t32, in_=wqkv[ch * P:(ch + 1) * P, :])
                t16 = consts.tile([P, 3 * C], BF16, tag=f"wqkv16{ch}")
                nc.vector.tensor_copy(t16, t32)
                wqkv16.append(t16)
            wft16 = []
            for ch in range(2):
                t32 = consts.tile([P, C], FP32, tag=f"wft32{ch}")
                nc.sync.dma_start(out=t32, in_=wft[ch * P:(ch + 1) * P, :])
                t16 = consts.tile([P, C], BF16, tag=f"wft16{ch}")
                nc.vector.tensor_copy(t16, t32)
                wft16.append(t16)
            vec_s = []
            for ch in range(2):
                t = consts.tile([P, 8], FP32, tag=f"vec{ch}")
                nc.sync.dma_start(out=t, in_=vecs[ch * P:(ch + 1) * P, :])
                vec_s.append(t)
            gsum_s = consts.tile([P, 16], FP32)
            nc.sync.dma_start(out=gsum_s, in_=gsum[:, :])
            gbc_s = consts.tile([16, P], FP32)
            nc.sync.dma_start(out=gbc_s, in_=gbc[:, :])
            ones_col = consts.tile([P, 1], BF16)
            nc.vector.memset(ones_col, 1.0)
            ones_row = consts.tile([1, P], FP32)
            nc.vector.memset(ones_row, 1.0)

            # ============ T1: temporal q/k/v projections (token-major) ========
            with tc.tile_pool(name="t1", bufs=3) as t1p, \
                 tc.tile_pool(name="t1ps", bufs=3, space="PSUM") as t1ps:
                GW = 512  # tokens per group (4 tiles of 128)
                g_order = [2 * t for t in range(T)] + [2 * t + 1 for t in range(T)]
                for g in g_order:
                    own = g < (FD // GW)
                    ncols = 3 * C if own else 2 * C
                    xt = t1p.tile([P, 2, GW], FP32, tag="xt")
                    for ch in range(2):
                        nc.sync.dma_start(
                            out=xt[:, ch, :],
                            in_=xin[ch * P:(ch + 1) * P, g * GW:(g + 1) * GW])
                    xt16 = t1p.tile([P, 2, GW], BF16, tag="xt16")
                    nc.scalar.activation(xt16, xt, AF.Copy)
                    kb = t1p.tile([P, 4, C], BF16, tag="kb")
                    vb = t1p.tile([P, 4, C], BF16, tag="vb")
                    qb = t1p.tile([P, 4, C], BF16, tag="qb", name="qb") if own else None
                    for s in range(4):
                        pp = t1ps.tile([P, 3 * C], FP32, tag="t1pp")
                        segs = ([(0, 512), (512, ncols)] if ncols > 512
                                else [(0, 512)])
                        for s0, s1 in segs:
                            for ch in range(2):
                                nc.tensor.matmul(
                                    pp[:, s0:s1],
                                    xt16[:, ch, s * P:(s + 1) * P],
                                    wqkv16[ch][:, s0:s1],
                                    start=(ch == 0), stop=(ch == 1))
                        nc.scalar.activation(kb[:, s], pp[:, 0:C], AF.Copy)
                        nc.vector.tensor_copy(vb[:, s], pp[:, C:2 * C])
                        if own:
                            nc.scalar.activation(qb[:, s], pp[:, 2 * C:3 * C],
                                                 AF.Copy, scale=inv_sqrt_dk)
                    kv_dst = k_t[g * GW:(g + 1) * GW, :]
                    nc.sync.dma_start(
                        out=kv_dst.rearrange("(s p) c -> p s c", s=4), in_=kb)
                    vv_dst = v_t[g * GW:(g + 1) * GW, :]
                    nc.sync.dma_start(
                        out=vv_dst.rearrange("(s p) c -> p s c", s=4), in_=vb)
                    if own:
                        qq_dst = q_t[g * GW:(g + 1) * GW, :]
                        nc.sync.dma_start(
                            out=qq_dst.rearrange("(s p) c -> p s c", s=4), in_=qb)

            # ============ T2: temporal attention (DVE) ============
            with tc.tile_pool(name="t2", bufs=2) as t2p:
                k_view = k_t.rearrange("(t lb p) c -> t lb p c", t=T, lb=NLB)
                v_view = v_t.rearrange("(t lb p) c -> t lb p c", t=T, lb=NLB)
                q_view = q_t.rearrange("(t lb p) c -> t lb p c", t=TOWN, lb=NLB)
                o_view = o_t.rearrange("(t lb p) c -> t lb p c", t=TOWN, lb=NLB)
                for lb in range(NLB):
                    kk = t2p.tile([P, T, C], BF16, tag="kk")
                    vv = t2p.tile([P, T, C], BF16, tag="vv")
                    qq = t2p.tile([P, TOWN, C], BF16, tag="qq")
                    nc.sync.dma_start(out=kk, in_=k_view[:, lb].transpose([1, 0, 2]))
                    nc.sync.dma_start(out=vv, in_=v_view[:, lb].transpose([1, 0, 2]))
                    nc.sync.dma_start(out=qq, in_=q_view[:, lb].transpose([1, 0, 2]))
                    # scores: prod[p, ti, tj, (n d)] = qq[p, ti, (n d)] * kk[p, tj, (n d)]
                    prod = t2p.tile([P, TOWN, T, C], BF16, tag="prodX", name="prod")
                    for ti in range(TOWN):
                        q_sl = qq[:, ti, :]  # [p, 256]
                        in0 = bass.AP(tensor=q_sl.tensor, offset=q_sl.offset,
                                      ap=[q_sl.ap[0], [0, T], q_sl.ap[1]])
                        nc.vector.tensor_tensor(out=prod[:, ti], in0=in0, in1=kk,
                                                op=OP.mult)
                    # S[p, (ti tj n)] = sum_d prod ; view prod as [p,(ti tj n),d]
                    s_sc = t2p.tile([P, TOWN, T, NH], BF16, tag="s_sc")
                    with nc.allow_low_precision(reason="scores are O(0.1)"):
                        nc.vector.tensor_reduce(
                            out=s_sc.rearrange("p a b n -> p (a b n)"),
                            in_=prod.rearrange("p a b (n d) -> p (a b n) d", n=NH),
                            axis=mybir.AxisListType.X, op=OP.add)
                    e_sc = t2p.tile([P, TOWN, T, NH], BF16, tag="e_sc")
                    nc.scalar.activation(e_sc, s_sc, AF.Exp)
                    # z[p, ti, n] = sum_tj e ; e viewed [p, (ti n), tj]
                    z = t2p.tile([P, TOWN, NH], FP32, tag="z")
                    for ti in range(TOWN):
                        e_sl = e_sc[:, ti]  # [p, T, NH]
                        nc.vector.tensor_reduce(
                            out=z[:, ti], in_=e_sl.transpose([0, 2, 1]),
                            axis=mybir.AxisListType.X, op=OP.add)
                    zr = t2p.tile([P, TOWN, NH], FP32, tag="zr")
                    nc.vector.reciprocal(out=zr.rearrange("p a n -> p (a n)"),
                                         in_=z.rearrange("p a n -> p (a n)"))
                    # AV: prod2[p, ti, (n d), tj] = e[p, ti, tj, n] * vv[p, tj, (n d)]
                    prod2 = t2p.tile([P, TOWN, C, T], BF16, tag="prodX", name="prod2")
                    vvv = vv.rearrange("p tj (n d) -> p tj n d", n=NH)
                    v_b = bass.AP(tensor=vvv.tensor, offset=vvv.offset,
                                  ap=[vvv.ap[0], vvv.ap[2], vvv.ap[3], vvv.ap[1]])
                    for ti in range(TOWN):
                        e_sl = e_sc[:, ti].rearrange("p tj n -> p n tj")
                        in0 = bass.AP(tensor=e_sl.tensor, offset=e_sl.offset,
                                      ap=[e_sl.ap[0], e_sl.ap[1], [0, DK],
                                          e_sl.ap[2]])
                        nc.vector.tensor_tensor(
                            out=prod2[:, ti].rearrange("p (n d) tj -> p n d tj",
                                                       n=NH),
                            in0=in0, in1=v_b, op=OP.mult)
                    oun = t2p.tile([P, TOWN, C], BF16, tag="oun")
                    with nc.allow_low_precision(reason="attn out is O(0.3)"):
                        nc.vector.tensor_reduce(
                            out=oun.rearrange("p a c -> p (a c)"),
                            in_=prod2.rearrange("p a c t -> p (a c) t"),
                            axis=mybir.AxisListType.X, op=OP.add)
                    # normalize by z (broadcast over d) and cast bf16
                    ob = t2p.tile([P, TOWN, C], BF16, tag="ob")
                    zv = zr  # [p, ti, n]
                    in1 = bass.AP(tensor=zv.tensor, offset=zv.offset,
                                  ap=[zv.ap[0], zv.ap[1], zv.ap[2], [0, DK]])
                    nc.vector.tensor_tensor(
                        out=ob.rearrange("p a (n d) -> p a n d", n=NH),
                        in0=oun.rearrange("p a (n d) -> p a n d", n=NH),
                        in1=in1, op=OP.mult)
                    nc.sync.dma_start(out=o_view[:, lb].transpose([1, 0, 2]), in_=ob)

            # ============ T3: temporal f-projection ============
            with tc.tile_pool(name="t3", bufs=3) as t3p, \
                 tc.tile_pool(name="t3ps", bufs=2, space="PSUM") as t3ps:
                for nt in range(FD // 512):
                    otr = [t3p.tile([P, 512], BF16, tag=f"otr{ch}", name=f"otr{ch}") for ch in range(2)]
                    for ch in range(2):
                        nc.sync.dma_start_transpose(
                            out=otr[ch],
                            in_=o_t[nt * 512:(nt + 1) * 512, ch * P:(ch + 1) * P])
                    for co in range(2):
                        ttp = t3ps.tile([P, 512], FP32, tag="ttp")
                        for ch in range(2):
                            nc.tensor.matmul(ttp, wft16[ch][:, co * P:(co + 1) * P],
                                             otr[ch], start=(ch == 0), stop=(ch == 1))
                        ttsb = t3p.tile([P, 512], FP32, tag="ttsb")
                        nc.scalar.activation(ttsb, ttp, AF.Identity,
                                             bias=vec_s[co][:, 6:7])
                        nc.sync.dma_start(
                            out=tt_t[co * P:(co + 1) * P, nt * 512:(nt + 1) * 512],
                            in_=ttsb)

            # ============ S: spatial attention per owned frame ============
            with tc.tile_pool(name="sp", bufs=2) as spp, \
                 tc.tile_pool(name="spb", bufs=1) as spb, \
                 tc.tile_pool(name="sps", bufs=2, space="PSUM") as sps, \
                 tc.tile_pool(name="spsb", bufs=1, space="PSUM") as spsb:
                for fr in range(TOWN):
                    col0 = fr * HWN
                    xf = [spp.tile([P, HWN], FP32, tag=f"xf{ch}", name=f"xf{ch}") for ch in range(2)]
                    for ch in range(2):
                        nc.sync.dma_start(
                            out=xf[ch], in_=xin[ch * P:(ch + 1) * P, col0:col0 + HWN])
                    # ---- GroupNorm stats ----
                    stats = spp.tile([P, 2, 2, 6], FP32, tag="stats")
                    mv = spp.tile([P, 2, 2], FP32, tag="mv")
                    for ch in range(2):
                        for sb_ in range(2):
                            nc.vector.bn_stats(out=stats[:, ch, sb_],
                                               in_=xf[ch][:, sb_ * 512:(sb_ + 1) * 512])
                        nc.vector.bn_aggr(out=mv[:, ch], in_=stats[:, ch])
                    # per-channel [mean, E[x^2]]
                    ex2 = spp.tile([P, 2, 2], FP32, tag="ex2")
                    for ch in range(2):
                        nc.vector.tensor_tensor(out=ex2[:, ch, 1:2],
                                                in0=mv[:, ch, 0:1], in1=mv[:, ch, 0:1],
                                                op=OP.mult)
                        nc.vector.tensor_tensor(out=ex2[:, ch, 1:2],
                                                in0=mv[:, ch, 1:2], in1=ex2[:, ch, 1:2],
                                                op=OP.add)
                        nc.vector.tensor_copy(ex2[:, ch, 0:1], mv[:, ch, 0:1])
                    # group sums via 0/1 matmul: [16, 2] per chunk
                    gstat = [sps.tile([16, 2], FP32, tag="pj", name=f"gstat{ch}")
                             for ch in range(2)]
                    for ch in range(2):
                        nc.tensor.matmul(gstat[ch], gsum_s, ex2[:, ch],
                                         start=True, stop=True)
                    gsb = spp.tile([16, 2, 2], FP32, tag="gsb")
                    for ch in range(2):
                        nc.vector.tensor_scalar_mul(gsb[:, ch], gstat[ch], 1.0 / CPG)
                    # catstat[:, ch, 0] = mean_g ; [:, ch, 1] = rstd_g
                    catstat = spp.tile([16, 2, 2], FP32, tag="catstat")
                    for ch in range(2):
                        nc.vector.tensor_copy(catstat[:, ch, 0:1], gsb[:, ch, 0:1])
                        # var = E[x^2] - mean^2
                        nc.vector.tensor_tensor(out=catstat[:, ch, 1:2],
                                                in0=gsb[:, ch, 0:1],
                                                in1=gsb[:, ch, 0:1], op=OP.mult)
                        nc.vector.tensor_tensor(out=catstat[:, ch, 1:2],
                                                in0=gsb[:, ch, 1:2],
                                                in1=catstat[:, ch, 1:2],
                                                op=OP.subtract)
                    # rstd = 1/sqrt(var + eps)
                    nc.vector.tensor_scalar_add(
                        catstat[:, :, 1], catstat[:, :, 1], EPS)
                    nc.scalar.activation(
                        catstat[:, :, 1], catstat[:, :, 1], AF.Sqrt)
                    nc.vector.reciprocal(out=catstat[:, :, 1], in_=catstat[:, :, 1])
                    # broadcast groups -> channels via 0/1 matmul [128, 2] per chunk
                    bc = [sps.tile([P, 2], FP32, tag="pj", name=f"bc{ch}")
                          for ch in range(2)]
                    for ch in range(2):
                        nc.tensor.matmul(bc[ch], gbc_s, catstat[:, ch],
                                         start=True, stop=True)
                    # scale_c = rstd*gamma ; bias_c = beta - mean*scale
                    scb = spp.tile([P, 2, 2], FP32, tag="scb")
                    for ch in range(2):
                        nc.vector.tensor_tensor(out=scb[:, ch, 0:1],
                                                in0=bc[ch][:, 1:2],
                                                in1=vec_s[ch][:, 0:1], op=OP.mult)
                        nc.vector.tensor_tensor(out=scb[:, ch, 1:2],
                                                in0=bc[ch][:, 0:1],
                                                in1=scb[:, ch, 0:1], op=OP.mult)
                        nc.vector.tensor_tensor(out=scb[:, ch, 1:2],
                                                in0=vec_s[ch][:, 1:2],
                                                in1=scb[:, ch, 1:2], op=OP.subtract)
                    hf = [spp.tile([P, HWN], BF16, tag=f"hf{ch}", name=f"hf{ch}") for ch in range(2)]
                    for ch in range(2):
                        nc.scalar.activation(
                            hf[ch], xf[ch], AF.Identity,
                            scale=scb[:, ch, 0:1], bias=scb[:, ch, 1:2])
                    # ---- q, k projections [co, hw] ----
                    qs = [spp.tile([P, HWN], BF16, tag=f"qs{ch}", name=f"qs{ch}") for ch in range(2)]
                    ks = [spp.tile([P, HWN], BF16, tag=f"ks{ch}", name=f"ks{ch}") for ch in range(2)]
                    for co in range(2):
                        for half in range(2):
                            nsl = slice(half * 512, (half + 1) * 512)
                            qp = sps.tile([P, 512], FP32, tag="pj")
                            kp = sps.tile([P, 512], FP32, tag="pj")
                            for ch in range(2):
                                nc.tensor.matmul(
                                    qp, wqsT_s[ch][:, co * P:(co + 1) * P],
                                    hf[ch][:, nsl],
                                    start=(ch == 0), stop=(ch == 1))
                                nc.tensor.matmul(
                                    kp, wksT_s[ch][:, co * P:(co + 1) * P],
                                    hf[ch][:, nsl],
                                    start=(ch == 0), stop=(ch == 1))
                            nc.scalar.activation(qs[co][:, nsl], qp, AF.Identity,
                                                 bias=vec_s[co][:, 2:3])
                            nc.scalar.activation(ks[co][:, nsl], kp, AF.Identity,
                                                 bias=vec_s[co][:, 3:4])
                    # ---- vT[hw, co] (bias bv folded in after AV) ----
                    vT = spb.tile([P, 8, C], BF16, tag="vT")
                    for hwt in range(8):
                        vp = sps.tile([P, C], FP32, tag="pj")
                        for ch in range(2):
                            nc.tensor.matmul(
                                vp, hf[ch][:, hwt * P:(hwt + 1) * P],
                                wvsT_s[ch],
                                start=(ch == 0), stop=(ch == 1))
                        nc.scalar.activation(vT[:, hwt], vp, AF.Copy)
                    # ---- scores S'[kj, qi] -> exp ----
                    ebuf = spb.tile([P, 8, HWN], BF16, tag="ebuf")
                    for kjt in range(8):
                        sp_ = sps.tile([P, HWN], FP32, tag="big")
                        for half in range(2):
                            nsl = slice(half * 512, (half + 1) * 512)
                            for ch in range(2):
                                nc.tensor.matmul(
                                    sp_[:, nsl],
                                    ks[ch][:, kjt * P:(kjt + 1) * P],
                                    qs[ch][:, nsl],
                                    start=(ch == 0), stop=(ch == 1))
                        nc.scalar.activation(ebuf[:, kjt], sp_, AF.Exp, scale=c_scale)
                    # ---- rowsum over kj via ones-matmul ----
                    rs = spsb.tile([1, HWN], FP32, tag="rs")
                    for half in range(2):
                        nsl = slice(half * 512, (half + 1) * 512)
                        for kjt in range(8):
                            nc.tensor.matmul(rs[:, nsl], ones_col, ebuf[:, kjt, nsl],
                                             start=(kjt == 0), stop=(kjt == 7))
                    rsr = spp.tile([1, HWN], FP32, tag="rsr")
                    nc.vector.reciprocal(out=rsr, in_=rs)
                    # broadcast rowsum recip to all partitions via K=1 matmul
                    rsbp = sps.tile([P, HWN], FP32, tag="big")
                    for half in range(2):
                        nsl = slice(half * 512, (half + 1) * 512)
                        nc.tensor.matmul(rsbp[:, nsl], ones_row, rsr[:, nsl],
                                         start=True, stop=True)
                    rsb = spp.tile([P, HWN], FP32, tag="rsb")
                    nc.scalar.activation(rsb, rsbp, AF.Copy)
                    # ---- AV + normalize + bv ----
                    attn_out = [spp.tile([P, HWN], BF16, tag=f"ao{ch}", name=f"ao{ch}")
                                for ch in range(2)]
                    for co in range(2):
                        avp = sps.tile([P, HWN], FP32, tag="big")
                        for half in range(2):
                            nsl = slice(half * 512, (half + 1) * 512)
                            for kjt in range(8):
                                nc.tensor.matmul(avp[:, nsl],
                                                 vT[:, kjt, co * P:(co + 1) * P],
                                                 ebuf[:, kjt, nsl],
                                                 start=(kjt == 0), stop=(kjt == 7))
                        nc.vector.tensor_tensor(out=attn_out[co], in0=avp, in1=rsb,
                                                op=OP.mult)
                        nc.vector.tensor_scalar_add(attn_out[co], attn_out[co],
                                                    vec_s[co][:, 4:5])
                    # ---- o-proj + residual + temporal ----
                    for co in range(2):
                        for half in range(2):
                            nsl = slice(half * 512, (half + 1) * 512)
                            op_ = sps.tile([P, 512], FP32, tag="pj")
                            for ch in range(2):
                                nc.tensor.matmul(
                                    op_,
                                    wosT_s[ch][:, co * P:(co + 1) * P],
                                    attn_out[ch][:, nsl],
                                    start=(ch == 0), stop=(ch == 1))
                            fin = spp.tile([P, 512], FP32, tag="fin")
                            nc.scalar.activation(fin, op_, AF.Identity,
                                                 bias=vec_s[co][:, 5:6])
                            nc.vector.tensor_tensor(out=fin, in0=fin,
                                                    in1=xf[co][:, nsl], op=OP.add)
                            nc.sync.dma_start(
                                out=s_scr[fr, co * P:(co + 1) * P, nsl], in_=fin)
            # ============ S-B: final add s + temporal ============
            with tc.tile_pool(name="sb2", bufs=3) as sb2:
                for fr in range(TOWN):
                    col0 = fr * HWN
                    for co in range(2):
                        sload = sb2.tile([P, HWN], FP32, tag="sload")
                        tload = sb2.tile([P, HWN], FP32, tag="tload")
                        nc.sync.dma_start(out=sload,
                                          in_=s_scr[fr, co * P:(co + 1) * P, :])
                        nc.sync.dma_start(
                            out=tload,
                            in_=tt_t[co * P:(co + 1) * P, col0:col0 + HWN])
                        fin2 = sb2.tile([P, HWN], FP32, tag="fin2")
                        nc.vector.tensor_tensor(out=fin2, in0=sload, in1=tload,
                                                op=OP.add)
                        nc.sync.dma_start(out=out[fr, co * P:(co + 1) * P, :],
                                          in_=fin2)

    _split_multi_waits(nc)
    return nc


def _get_nc():
    if "nc" not in _CACHE:
        _CACHE["nc"] = _build_bass()
    return _CACHE["nc"]


def _make_in_maps(inputs):
    x = np.asarray(inputs["x"], dtype=np.float32)  # (B, C, T, H, W)
    g = {k: np.asarray(v, dtype=np.float32) for k, v in inputs.items()}

    wqkv = np.concatenate([g["wk_t"], g["wv_t"], g["wq_t"]], axis=1)  # [C, 3C]
    vecs = np.stack([g["gn_gamma"], g["gn_beta"], g["bq_s"], g["bk_s"],
                     g["bv_s"], g["bo_s"], g["bf_t"],
                     np.zeros(C, np.float32)], axis=1)  # [C, 8]
    pidx = np.arange(P)
    gsum = (pidx[:, None] // CPG == np.arange(16)[None, :]).astype(np.float32)
    gbc = np.ascontiguousarray(gsum.T)

    shared = dict(
        wqsT=np.ascontiguousarray(g["wq_s"].T), wksT=np.ascontiguousarray(g["wk_s"].T),
        wvsT=np.ascontiguousarray(g["wv_s"].T), wosT=np.ascontiguousarray(g["wo_s"].T),
        wqkv=np.ascontiguousarray(wqkv), wft=np.ascontiguousarray(g["wf_t"]),
        vecs=np.ascontiguousarray(vecs), gsum=gsum, gbc=gbc,
    )
    in_maps = []
    for core in range(8):
        b, fg = core // 4, core % 4
        xb = np.roll(x[b], -TOWN * fg, axis=1)  # (C, T, H, W), owned frames first
        m = dict(shared)
        m["x"] = np.ascontiguousarray(xb.reshape(C, NTOK))
        in_maps.append(m)
    return in_maps


def kernel(**inputs):
    in_maps = _make_in_maps(inputs)
    nc = _get_nc()
    res = run_bass_kernel_spmd(nc, in_maps, core_ids=list(range(8)))

    out = np.empty((B, C, T, H, W), dtype=np.float32)
    for core in range(8):
        b, fg = core // 4, core % 4
        o = res.results[core]["out"]  # [TOWN, C, HWN]
        out[b, :, TOWN * fg:TOWN * (fg + 1)] = (
            o.transpose(1, 0, 2).reshape(C, TOWN, H, W))
    return out
